# revision 14
# baseline (speedup 1.0000x reference)
"""Separable depthwise box filter (r=8, 'same' zero padding) on 8 trn2 cores.

Math: per (n, c) plane P (512x512), out = s^2 * (Bo @ P @ Bo) where Bo is the
symmetric banded 512x512 matrix of ONES with |i - j| <= r and s = 1/(2r+1).
Computing with a band of ones keeps Bo exact in bf16; the s^2 normalization is
folded into the pass-2 PSUM evacuation (fp32 scale, then bf16 cast).

On the PE (out = lhsT.T @ rhs):

  pass 1: Zt = matmul(lhsT=P,  rhs=Bo) = P.T @ Bo  (vertical filter, transposed)
  pass 2: Y  = matmul(lhsT=Zt, rhs=Bo) = Z  @ Bo   (horizontal filter, restored)

Everything on-chip is bf16 (inputs cast on host): fp32 matmuls run as HI/LO
pairs at 2x stream cost, so bf16 halves PE time AND halves HBM traffic.  PSUM
accumulates in fp32, so only the band sums see bf16 rounding.

Only the banded column windows of Bo are streamed AND loaded: the K-chunk of
rows [128a, 128a+128) of Bo has nonzero columns only in [128a-r, 128a+128+r).
PSUM's per-element has_written bit makes the overlapping column windows
accumulate while fresh columns overwrite.

Pipeline structure (per core, 16 planes):
  - planes are software-pipelined: pass1(p+1) is emitted between pass1(p) and
    pass2(p), so the PE fills the Z-evacuation latency with the next plane's
    pass-1 matmuls instead of stalling (keeps HAM un-throttled, 2.4 GHz).
  - PSUM tiles are bank PAIRS ([128, 2, 512] f32): one evac op per 2 banks
    halves the fixed per-op cost and semaphore traffic on DVE/ACT.
  - evacs alternate DVE/ACT; input loads ride the SP HWDGE ring, output
    stores the GpSimd SWDGE queue, so in/out flow on separate DMA queues and
    cost no DVE/ACT engine time.

Sharding: batch dim (8) across the 8 cores; each core filters its 16 channel
planes independently (no cross-core communication).
"""

import numpy as np

_CACHE = {}

N_CORES = 8
P = 128
H = W = 512
A = H // P  # 4 row-chunks per plane


def _band_windows(r):
    """Nonzero column window [n0, n1) of Bo rows [128a, 128a+128), per a."""
    return [(max(0, P * a - r), min(W, P * a + P + r)) for a in range(A)]


def _build(r, n_planes):
    import concourse.mybir as mybir
    from concourse import bacc
    from concourse.tile import TileContext

    bf16 = mybir.dt.bfloat16
    f32 = mybir.dt.float32
    win = _band_windows(r)
    wmax = max(n1 - n0 for n0, n1 in win)
    inv_k2 = float(1.0 / float(2 * r + 1) ** 2)

    nc = bacc.Bacc()
    x_d = nc.declare_dram_parameter("x", [n_planes * H, W], bf16, isOutput=False)
    # b holds only the banded windows, packed: row-chunk a's window in b[:, a, :]
    b_d = nc.declare_dram_parameter("b", [P, A * wmax], bf16, isOutput=False)
    y_d = nc.declare_dram_parameter("y", [n_planes * H, W], bf16, isOutput=True)

    x_ap = x_d.ap().rearrange("(p a q) n -> p q a n", p=n_planes, q=P)
    y_ap = y_d.ap().rearrange("(p a q) n -> p q a n", p=n_planes, q=P)
    b_ap = b_d.ap().rearrange("q (a n) -> q a n", a=A)

    with TileContext(nc) as tc:
        with (
            tc.tile_pool(name="bmat", bufs=1) as bpool,
            tc.tile_pool(name="xin", bufs=5) as xpool,
            tc.tile_pool(name="zmid", bufs=3) as zpool,
            tc.tile_pool(name="yout", bufs=4) as opool,
            tc.tile_pool(name="ps", bufs=4, space="PSUM") as psp,
        ):
            bt = bpool.tile([P, A, wmax], bf16)
            xt0 = xpool.tile([P, A, W], bf16, name="xt0", tag="xt")
            # One DMA for all Bo windows (0.14 MB), then plane 0 in
            # chunks: the first matmul group only needs Bo + chunk a=0, so
            # the PE starts ~3us earlier than with a whole-plane load.
            nc.sync.dma_start(out=bt[:], in_=b_ap[:])
            for a in range(A):
                nc.sync.dma_start(out=xt0[:, a, :], in_=x_ap[0, :, a, :])

            xts = {0: xt0}
            zts = {}

            def pass1(p):
                if p not in xts:
                    xt = xpool.tile([P, A, W], bf16, name="xt", tag="xt")
                    nc.sync.dma_start(out=xt[:], in_=x_ap[p])
                    xts[p] = xt
                xt = xts[p]
                zt = zpool.tile([P, A, W], bf16)
                zts[p] = zt
                for half in range(2):
                    ps = psp.tile([P, 2, W], f32, name="ps1", tag="ps")
                    for mm in range(2):
                        m = 2 * half + mm
                        for a in range(A):
                            n0, n1 = win[a]
                            nc.tensor.matmul(
                                ps[:, mm, n0:n1],
                                xt[:, a, m * P : (m + 1) * P],
                                bt[:, a, : n1 - n0],
                                start=(a == 0),
                                stop=(a == A - 1),
                                skip_group_check=True,
                            )
                    # both pass-1 evacs on DVE: each engine's ops then
                    # complete in PE order, avoiding head-of-line sem waits
                    nc.vector.tensor_copy(out=zt[:, 2 * half : 2 * half + 2, :], in_=ps[:])
                del xts[p]

            def pass2(p):
                zt = zts.pop(p)
                ot = opool.tile([P, A, W], bf16)
                for half in range(2):
                    ps = psp.tile([P, 2, W], f32, name="ps2", tag="ps")
                    for mm in range(2):
                        m = 2 * half + mm
                        for a in range(A):
                            n0, n1 = win[a]
                            nc.tensor.matmul(
                                ps[:, mm, n0:n1],
                                zt[:, a, m * P : (m + 1) * P],
                                bt[:, a, : n1 - n0],
                                start=(a == 0),
                                stop=(a == A - 1),
                                skip_group_check=True,
                            )
                    # pass-2 evacs (with the 1/(2r+1)^2 scale) on ACT
                    nc.scalar.mul(ot[:, 2 * half : 2 * half + 2, :], ps[:], inv_k2)
                    # last two planes store via the ACT HWDGE ring so the
                    # POOL SWDGE queue is already drained at kernel end (its
                    # final DRAIN barrier otherwise costs ~4us)
                    eng = nc.scalar if p >= n_planes - 2 else nc.gpsimd
                    eng.dma_start(
                        out=y_ap[p, :, 2 * half : 2 * half + 2, :],
                        in_=ot[:, 2 * half : 2 * half + 2, :],
                    )

            # depth-3 software pipeline: pass1 runs TWO planes ahead of pass2,
            # so pass2(p)'s zt dependency has a full plane of slack and the
            # PE never stalls on the pass-1 evacuations.
            pass1(0)
            pass1(1)
            for p in range(n_planes):
                if p + 2 < n_planes:
                    pass1(p + 2)
                pass2(p)

    # Drop the preamble's GpSimd memsets of unused const tiles: Q7 memsets
    # cost ~us each and gate the post-preamble all-engine barrier, delaying
    # kernel start.  Keep any const a later instruction actually reads.
    used = set()
    for bb in nc.main_func.blocks:
        for inst in bb.instructions:
            if type(inst).__name__ == "InstMemset":
                continue
            for ap in list(inst.ins or []) + list(inst.outs or []):
                ref = getattr(ap, "memref", None)
                if ref and str(ref).startswith("const-"):
                    used.add(str(ref))
    entry = nc.main_func.blocks[0]
    dropped = [
        inst
        for inst in entry.instructions
        if type(inst).__name__ == "InstMemset"
        and inst.outs
        and str(getattr(inst.outs[0], "memref", "")).startswith("const-")
        and str(inst.outs[0].memref) not in used
    ]
    for inst in dropped:
        entry.instructions.remove(inst)

    nc.finalize()
    return nc


def _band_windows_payload(r):
    """Packed nonzero windows of the band-of-ones matrix, [P, A*wmax] f32."""
    win = _band_windows(r)
    wmax = max(n1 - n0 for n0, n1 in win)
    out = np.zeros((P, A * wmax), dtype=np.float32)
    for a, (n0, n1) in enumerate(win):
        for q in range(P):
            i = P * a + q  # absolute row of Bo
            lo = max(n0, i - r, 0)
            hi = min(n1, i + r + 1, W)
            out[q, a * wmax + (lo - n0) : a * wmax + (hi - n0)] = 1.0
    return out


def kernel(x, r):
    import ml_dtypes
    from concourse.bass_utils import run_bass_kernel_spmd

    r = int(r)
    x = np.asarray(x)
    n, c, h, w = x.shape
    assert (h, w) == (H, W) and n == N_CORES, (n, c, h, w)

    key = (r, c)
    if key not in _CACHE:
        _CACHE[key] = _build(r, c)
    nc = _CACHE[key]

    bf16 = ml_dtypes.bfloat16
    xb = np.ascontiguousarray(x.reshape(n, c * H, W)).astype(bf16)
    b = _band_windows_payload(r).astype(bf16)
    in_maps = [{"x": xb[i], "b": b} for i in range(n)]
    res = run_bass_kernel_spmd(nc, in_maps, core_ids=list(range(N_CORES)))
    out = np.stack(
        [np.asarray(res.results[i]["y"]).astype(np.float32).reshape(c, H, W) for i in range(n)]
    )
    return out


# revision 17
# speedup vs baseline: 1.0118x; 1.0118x over previous
"""Separable depthwise box filter (r=8, 'same' zero padding) on 8 trn2 cores.

Math: per (n, c) plane P (512x512), out = s^2 * (Bo @ P @ Bo) where Bo is the
symmetric banded 512x512 matrix of ONES with |i - j| <= r and s = 1/(2r+1).
Computing with a band of ones keeps Bo exact in bf16; the s^2 normalization is
folded into the pass-2 PSUM evacuation (fp32 scale, then bf16 cast).

On the PE (out = lhsT.T @ rhs):

  pass 1: Zt = matmul(lhsT=P,  rhs=Bo) = P.T @ Bo  (vertical filter, transposed)
  pass 2: Y  = matmul(lhsT=Zt, rhs=Bo) = Z  @ Bo   (horizontal filter, restored)

Everything on-chip is bf16 (inputs cast on host): fp32 matmuls run as HI/LO
pairs at 2x stream cost, so bf16 halves PE time AND halves HBM traffic.  PSUM
accumulates in fp32, so only the band sums see bf16 rounding.

Only the banded column windows of Bo are streamed AND loaded: the K-chunk of
rows [128a, 128a+128) of Bo has nonzero columns only in [128a-r, 128a+128+r).
PSUM's per-element has_written bit makes the overlapping column windows
accumulate while fresh columns overwrite.

Pipeline structure (per core, 16 planes):
  - depth-3 software pipeline: pass1 runs TWO planes ahead of pass2, so the
    pass-2 dependency on the fully-evacuated Z has a whole plane of slack and
    the PE never stalls (keeps HAM un-throttled at 2.4 GHz).
  - PSUM tiles are bank PAIRS ([128, 2, 512] f32): one evac op per 2 banks
    halves the fixed per-op cost and semaphore traffic on DVE/ACT.
  - pass-1 evacs all on DVE, pass-2 evacs (with the scale) all on ACT: each
    engine's ops then complete in PE order, avoiding head-of-line sem waits.
  - input loads ride the SP HWDGE ring, output stores the GpSimd SWDGE queue
    (separate DMA queues, no DVE/ACT engine time); the last two planes store
    via the ACT HWDGE ring so the SWDGE queue is drained before the final
    end-of-kernel barrier.

Sharding: batch dim (8) across the 8 cores; each core filters its 16 channel
planes independently (no cross-core communication).
"""

import numpy as np

_CACHE = {}

N_CORES = 8
P = 128
H = W = 512
A = H // P  # 4 row-chunks per plane


def _band_windows(r):
    """Nonzero column window [n0, n1) of Bo rows [128a, 128a+128), per a."""
    return [(max(0, P * a - r), min(W, P * a + P + r)) for a in range(A)]


def _build(r, n_planes):
    import concourse.mybir as mybir
    from concourse import bacc
    from concourse.tile import TileContext

    bf16 = mybir.dt.bfloat16
    f32 = mybir.dt.float32
    win = _band_windows(r)
    wmax = max(n1 - n0 for n0, n1 in win)
    inv_k2 = float(1.0 / float(2 * r + 1) ** 2)

    nc = bacc.Bacc()
    x_d = nc.declare_dram_parameter("x", [n_planes * H, W], bf16, isOutput=False)
    # b holds only the banded windows, packed: row-chunk a's window in b[:, a, :]
    b_d = nc.declare_dram_parameter("b", [P, A * wmax], bf16, isOutput=False)
    y_d = nc.declare_dram_parameter("y", [n_planes * H, W], bf16, isOutput=True)

    x_ap = x_d.ap().rearrange("(p a q) n -> p q a n", p=n_planes, q=P)
    y_ap = y_d.ap().rearrange("(p a q) n -> p q a n", p=n_planes, q=P)
    b_ap = b_d.ap().rearrange("q (a n) -> q a n", a=A)

    with TileContext(nc) as tc:
        with (
            tc.tile_pool(name="bmat", bufs=1) as bpool,
            tc.tile_pool(name="xin", bufs=5) as xpool,
            tc.tile_pool(name="zmid", bufs=3) as zpool,
            tc.tile_pool(name="yout", bufs=4) as opool,
            tc.tile_pool(name="ps", bufs=4, space="PSUM") as psp,
        ):
            bt = bpool.tile([P, A, wmax], bf16)
            xt0 = xpool.tile([P, A, W], bf16, name="xt0", tag="xt")
            # One DMA for all Bo windows (0.14 MB), then plane 0: few
            # SP issue slots before the steady-state prefetch begins.
            nc.sync.dma_start(out=bt[:], in_=b_ap[:])
            nc.sync.dma_start(out=xt0[:], in_=x_ap[0])

            xts = {0: xt0}
            zts = {}

            def pass1(p):
                if p not in xts:
                    xt = xpool.tile([P, A, W], bf16, name="xt", tag="xt")
                    nc.sync.dma_start(out=xt[:], in_=x_ap[p])
                    xts[p] = xt
                xt = xts[p]
                zt = zpool.tile([P, A, W], bf16)
                zts[p] = zt
                for half in range(2):
                    ps = psp.tile([P, 2, W], f32, name="ps1", tag="ps")
                    for mm in range(2):
                        m = 2 * half + mm
                        for a in range(A):
                            n0, n1 = win[a]
                            nc.tensor.matmul(
                                ps[:, mm, n0:n1],
                                xt[:, a, m * P : (m + 1) * P],
                                bt[:, a, : n1 - n0],
                                start=(a == 0),
                                stop=(a == A - 1),
                                skip_group_check=True,
                            )
                    # both pass-1 evacs on DVE: each engine's ops then
                    # complete in PE order, avoiding head-of-line sem waits
                    nc.vector.tensor_copy(out=zt[:, 2 * half : 2 * half + 2, :], in_=ps[:])
                del xts[p]

            def pass2(p):
                zt = zts.pop(p)
                ot = opool.tile([P, A, W], bf16)
                for half in range(2):
                    ps = psp.tile([P, 2, W], f32, name="ps2", tag="ps")
                    for mm in range(2):
                        m = 2 * half + mm
                        for a in range(A):
                            n0, n1 = win[a]
                            nc.tensor.matmul(
                                ps[:, mm, n0:n1],
                                zt[:, a, m * P : (m + 1) * P],
                                bt[:, a, : n1 - n0],
                                start=(a == 0),
                                stop=(a == A - 1),
                                skip_group_check=True,
                            )
                    # pass-2 evacs (with the 1/(2r+1)^2 scale) on ACT
                    nc.scalar.mul(ot[:, 2 * half : 2 * half + 2, :], ps[:], inv_k2)
                    # last two planes store via the ACT HWDGE ring so the
                    # POOL SWDGE queue is already drained at kernel end (its
                    # final DRAIN barrier otherwise costs ~4us)
                    eng = nc.scalar if p >= n_planes - 2 else nc.gpsimd
                    eng.dma_start(
                        out=y_ap[p, :, 2 * half : 2 * half + 2, :],
                        in_=ot[:, 2 * half : 2 * half + 2, :],
                    )

            # depth-3 software pipeline: pass1 runs TWO planes ahead of pass2,
            # so pass2(p)'s zt dependency has a full plane of slack and the
            # PE never stalls on the pass-1 evacuations.
            pass1(0)
            pass1(1)
            for p in range(n_planes):
                if p + 2 < n_planes:
                    pass1(p + 2)
                pass2(p)

    # Hoist the first SP input DMAs (Bo windows + planes 0-1) ahead of the
    # all-engine rendezvous in the entry block.  SP reaches the entry block at
    # ~0.1us while the rendezvous only releases at ~3.4us (gated by the PE's
    # runtime-init stall), and walrus inserts its library-load prologue after
    # it -- so without the hoist the first data lands ~9us in.  The hoisted
    # DMAs carry no semaphore waits (verified via concise()), only DMAHW
    # completion increments, and the runtime's fenced sem-clear completes
    # before GpSimd reaches the entry block, so the early increments survive.
    entry_bb = nc.main_func.blocks[0]
    body_bb = nc.main_func.blocks[1]
    sp_drain_idx = next(
        i
        for i, inst in enumerate(entry_bb.instructions)
        if type(inst).__name__ == "InstDrain"
        and str(getattr(inst, "engine", "")).endswith("SP")
    )
    hoist = []
    for inst in list(body_bb.instructions):
        if len(hoist) >= 3:
            break
        if type(inst).__name__ == "InstDMACopy" and str(
            getattr(inst, "engine", "")
        ).endswith("SP"):
            hoist.append(inst)
            body_bb.instructions.remove(inst)
    for off, inst in enumerate(hoist):
        entry_bb.instructions.insert(sp_drain_idx + off, inst)

    # Drop the preamble's GpSimd memsets of unused const tiles: Q7 memsets
    # cost ~us each and gate the post-preamble all-engine barrier, delaying
    # kernel start.  Keep any const a later instruction actually reads.
    used = set()
    for bb in nc.main_func.blocks:
        for inst in bb.instructions:
            if type(inst).__name__ == "InstMemset":
                continue
            for ap in list(inst.ins or []) + list(inst.outs or []):
                ref = getattr(ap, "memref", None)
                if ref and str(ref).startswith("const-"):
                    used.add(str(ref))
    entry = nc.main_func.blocks[0]
    dropped = [
        inst
        for inst in entry.instructions
        if type(inst).__name__ == "InstMemset"
        and inst.outs
        and str(getattr(inst.outs[0], "memref", "")).startswith("const-")
        and str(inst.outs[0].memref) not in used
    ]
    for inst in dropped:
        entry.instructions.remove(inst)

    nc.finalize()
    return nc


def _band_windows_payload(r):
    """Packed nonzero windows of the band-of-ones matrix, [P, A*wmax] f32."""
    win = _band_windows(r)
    wmax = max(n1 - n0 for n0, n1 in win)
    out = np.zeros((P, A * wmax), dtype=np.float32)
    for a, (n0, n1) in enumerate(win):
        for q in range(P):
            i = P * a + q  # absolute row of Bo
            lo = max(n0, i - r, 0)
            hi = min(n1, i + r + 1, W)
            out[q, a * wmax + (lo - n0) : a * wmax + (hi - n0)] = 1.0
    return out


def kernel(x, r):
    import ml_dtypes
    from concourse.bass_utils import run_bass_kernel_spmd

    r = int(r)
    x = np.asarray(x)
    n, c, h, w = x.shape
    assert (h, w) == (H, W) and n == N_CORES, (n, c, h, w)

    key = (r, c)
    if key not in _CACHE:
        _CACHE[key] = _build(r, c)
    nc = _CACHE[key]

    bf16 = ml_dtypes.bfloat16
    xb = np.ascontiguousarray(x.reshape(n, c * H, W)).astype(bf16)
    b = _band_windows_payload(r).astype(bf16)
    in_maps = [{"x": xb[i], "b": b} for i in range(n)]
    res = run_bass_kernel_spmd(nc, in_maps, core_ids=list(range(N_CORES)))
    out = np.stack(
        [np.asarray(res.results[i]["y"]).astype(np.float32).reshape(c, H, W) for i in range(n)]
    )
    return out


# revision 19
# speedup vs baseline: 1.1374x; 1.1241x over previous
"""Separable depthwise box filter (r=8, 'same' zero padding) on 8 trn2 cores.

Math: per (n, c) plane P (512x512), out = s^2 * (Bo @ P @ Bo) where Bo is the
symmetric banded 512x512 matrix of ONES with |i - j| <= r and s = 1/(2r+1).
Computing with a band of ones keeps Bo exact in bf16; the s^2 normalization is
folded into the pass-2 PSUM evacuation (fp32 scale, then bf16 cast).

On the PE (out = lhsT.T @ rhs):

  pass 1: Zt = matmul(lhsT=P,  rhs=Bo) = P.T @ Bo  (vertical filter, transposed)
  pass 2: Y  = matmul(lhsT=Zt, rhs=Bo) = Z  @ Bo   (horizontal filter, restored)

Everything on-chip is bf16 (inputs cast on host): fp32 matmuls run as HI/LO
pairs at 2x stream cost, so bf16 halves PE time AND halves HBM traffic.  PSUM
accumulates in fp32, so only the band sums see bf16 rounding.

Only the banded column windows of Bo are streamed AND loaded: the K-chunk of
rows [128a, 128a+128) of Bo has nonzero columns only in [128a-r, 128a+128+r).
PSUM's per-element has_written bit makes the overlapping column windows
accumulate while fresh columns overwrite.

Pipeline structure (per core, 16 planes):
  - depth-3 software pipeline: pass1 runs TWO planes ahead of pass2, so the
    pass-2 dependency on the fully-evacuated Z has a whole plane of slack and
    the PE never stalls (keeps HAM un-throttled at 2.4 GHz).
  - PSUM tiles are bank PAIRS ([128, 2, 512] f32): one evac op per 2 banks
    halves the fixed per-op cost and semaphore traffic on DVE/ACT.
  - pass-1 evacs all on DVE, pass-2 evacs (with the scale) all on ACT: each
    engine's ops then complete in PE order, avoiding head-of-line sem waits.
  - input loads ride the SP HWDGE ring, output stores the GpSimd SWDGE queue
    (separate DMA queues, no DVE/ACT engine time); the last two planes store
    via the ACT HWDGE ring so the SWDGE queue is drained before the final
    end-of-kernel barrier.

Sharding: batch dim (8) across the 8 cores; each core filters its 16 channel
planes independently (no cross-core communication).
"""

import numpy as np

_CACHE = {}

N_CORES = 8
P = 128
H = W = 512
A = H // P  # 4 row-chunks per plane


def _band_windows(r):
    """Nonzero column window [n0, n1) of Bo rows [128a, 128a+128), per a."""
    return [(max(0, P * a - r), min(W, P * a + P + r)) for a in range(A)]


def _build(r, n_planes):
    import concourse.mybir as mybir
    from concourse import bacc
    from concourse.tile import TileContext

    bf16 = mybir.dt.bfloat16
    f32 = mybir.dt.float32
    win = _band_windows(r)
    wmax = max(n1 - n0 for n0, n1 in win)
    inv_k2 = float(1.0 / float(2 * r + 1) ** 2)

    nc = bacc.Bacc()
    x_d = nc.declare_dram_parameter("x", [n_planes * H, W], bf16, isOutput=False)
    # b holds only the banded windows, packed: row-chunk a's window in b[:, a, :]
    b_d = nc.declare_dram_parameter("b", [P, A * wmax], bf16, isOutput=False)
    y_d = nc.declare_dram_parameter("y", [n_planes * H, W], bf16, isOutput=True)

    x_ap = x_d.ap().rearrange("(p a q) n -> p q a n", p=n_planes, q=P)
    y_ap = y_d.ap().rearrange("(p a q) n -> p q a n", p=n_planes, q=P)
    b_ap = b_d.ap().rearrange("q (a n) -> q a n", a=A)

    with TileContext(nc) as tc:
        with (
            tc.tile_pool(name="bmat", bufs=1) as bpool,
            tc.tile_pool(name="xin", bufs=5) as xpool,
            tc.tile_pool(name="zmid", bufs=3) as zpool,
            tc.tile_pool(name="yout", bufs=4) as opool,
            tc.tile_pool(name="ps", bufs=4, space="PSUM") as psp,
        ):
            bt = bpool.tile([P, A, wmax], bf16)
            xt0 = xpool.tile([P, A, W], bf16, name="xt0", tag="xt")
            # One DMA for all Bo windows (0.14 MB), then plane 0: few
            # SP issue slots before the steady-state prefetch begins.
            nc.sync.dma_start(out=bt[:], in_=b_ap[:])
            nc.sync.dma_start(out=xt0[:], in_=x_ap[0])

            xts = {0: xt0}
            zts = {}

            def pass1(p):
                if p not in xts:
                    xt = xpool.tile([P, A, W], bf16, name="xt", tag="xt")
                    nc.sync.dma_start(out=xt[:], in_=x_ap[p])
                    xts[p] = xt
                xt = xts[p]
                zt = zpool.tile([P, A, W], bf16)
                zts[p] = zt
                for half in range(2):
                    ps = psp.tile([P, 2, W], f32, name="ps1", tag="ps")
                    for mm in range(2):
                        m = 2 * half + mm
                        for a in range(A):
                            n0, n1 = win[a]
                            nc.tensor.matmul(
                                ps[:, mm, n0:n1],
                                xt[:, a, m * P : (m + 1) * P],
                                bt[:, a, : n1 - n0],
                                start=(a == 0),
                                stop=(a == A - 1),
                                skip_group_check=True,
                            )
                    # both pass-1 evacs on DVE: each engine's ops then
                    # complete in PE order, avoiding head-of-line sem waits
                    nc.vector.tensor_copy(out=zt[:, 2 * half : 2 * half + 2, :], in_=ps[:])
                del xts[p]

            def pass2(p):
                zt = zts.pop(p)
                ot = opool.tile([P, A, W], bf16)
                for half in range(2):
                    ps = psp.tile([P, 2, W], f32, name="ps2", tag="ps")
                    for mm in range(2):
                        m = 2 * half + mm
                        for a in range(A):
                            n0, n1 = win[a]
                            nc.tensor.matmul(
                                ps[:, mm, n0:n1],
                                zt[:, a, m * P : (m + 1) * P],
                                bt[:, a, : n1 - n0],
                                start=(a == 0),
                                stop=(a == A - 1),
                                skip_group_check=True,
                            )
                    # pass-2 evacs (with the 1/(2r+1)^2 scale) on ACT
                    nc.scalar.mul(ot[:, 2 * half : 2 * half + 2, :], ps[:], inv_k2)
                    # last two planes store via the SP HWDGE ring (warm
                    # from the input loads, and idle by then) so the POOL
                    # SWDGE queue is already drained at kernel end (its final
                    # DRAIN barrier otherwise costs ~4us)
                    eng = nc.sync if p >= n_planes - 2 else nc.gpsimd
                    eng.dma_start(
                        out=y_ap[p, :, 2 * half : 2 * half + 2, :],
                        in_=ot[:, 2 * half : 2 * half + 2, :],
                    )

            # depth-3 software pipeline: pass1 runs TWO planes ahead of pass2,
            # so pass2(p)'s zt dependency has a full plane of slack and the
            # PE never stalls on the pass-1 evacuations.
            pass1(0)
            pass1(1)
            for p in range(n_planes):
                if p + 2 < n_planes:
                    pass1(p + 2)
                pass2(p)

    # Drop the preamble's GpSimd memsets of unused const tiles: Q7 memsets
    # cost ~us each and gate the post-preamble all-engine barrier, delaying
    # kernel start.  Keep any const a later instruction actually reads.
    used = set()
    for bb in nc.main_func.blocks:
        for inst in bb.instructions:
            if type(inst).__name__ == "InstMemset":
                continue
            for ap in list(inst.ins or []) + list(inst.outs or []):
                ref = getattr(ap, "memref", None)
                if ref and str(ref).startswith("const-"):
                    used.add(str(ref))
    entry = nc.main_func.blocks[0]
    dropped = [
        inst
        for inst in entry.instructions
        if type(inst).__name__ == "InstMemset"
        and inst.outs
        and str(getattr(inst.outs[0], "memref", "")).startswith("const-")
        and str(inst.outs[0].memref) not in used
    ]
    for inst in dropped:
        entry.instructions.remove(inst)

    nc.finalize()
    return nc


def _band_windows_payload(r):
    """Packed nonzero windows of the band-of-ones matrix, [P, A*wmax] f32."""
    win = _band_windows(r)
    wmax = max(n1 - n0 for n0, n1 in win)
    out = np.zeros((P, A * wmax), dtype=np.float32)
    for a, (n0, n1) in enumerate(win):
        for q in range(P):
            i = P * a + q  # absolute row of Bo
            lo = max(n0, i - r, 0)
            hi = min(n1, i + r + 1, W)
            out[q, a * wmax + (lo - n0) : a * wmax + (hi - n0)] = 1.0
    return out


def kernel(x, r):
    import ml_dtypes
    from concourse.bass_utils import run_bass_kernel_spmd

    r = int(r)
    x = np.asarray(x)
    n, c, h, w = x.shape
    assert (h, w) == (H, W) and n == N_CORES, (n, c, h, w)

    key = (r, c)
    if key not in _CACHE:
        _CACHE[key] = _build(r, c)
    nc = _CACHE[key]

    bf16 = ml_dtypes.bfloat16
    xb = np.ascontiguousarray(x.reshape(n, c * H, W)).astype(bf16)
    b = _band_windows_payload(r).astype(bf16)
    in_maps = [{"x": xb[i], "b": b} for i in range(n)]
    res = run_bass_kernel_spmd(nc, in_maps, core_ids=list(range(N_CORES)))
    out = np.stack(
        [np.asarray(res.results[i]["y"]).astype(np.float32).reshape(c, H, W) for i in range(n)]
    )
    return out
